# revision 6
# baseline (speedup 1.0000x reference)
"""VQ nearest-embedding kernel for Trainium2 (8 NeuronCores, data-parallel).

Problem: x (64,256,32,32) f32, emb (256,1024) f32.
  xf = x.transpose(0,2,3,1).reshape(-1,256)       # (65536, 256)
  argmin_k ||xf - emb[:,k]||^2  ==  argmax_k s,  s = xf @ emb - 0.5*||emb_k||^2
  out = emb[:, argmin] scattered back to (64,256,32,32); also return argmin (64,32,32) i32.

Sharding: batch across 8 cores (8 items each). Codebook replicated.

Per-core device pipeline, per 128-hw-position tile (64 tiles):
  SP DMA: xg0/xg1 (d-chunks of x, natively the transposed lhsT layout)
  ACT: prefill PSUM (128,1024) with bias tile -0.5*||e_k||^2 broadcast
  PE: 4x fp32r matmuls accumulate onto the bias (s in PSUM)
  DVE: reduce_max -> m; scalar_tensor_tensor is_ge(s,m)*iota sum-accum -> argmax idx
  POOL: indirect-DMA gather embT[idx] -> (128n, 256d)
  PE: 2x fp32 transposes -> (128d, 128n); ACT copies PSUM->SBUF; SP DMA to out
"""
import sys
sys.path.insert(0, "/opt/trn_rl_repo")
import numpy as np
from contextlib import ExitStack

import concourse.bass as bass
import concourse.bacc as bacc
import concourse.mybir as mybir
import concourse.tile as tile
from concourse.masks import make_identity
from concourse.bass_utils import run_bass_kernel_spmd

F32 = mybir.dt.float32
F32R = mybir.dt.float32r
F16 = mybir.dt.float16
I32 = mybir.dt.int32

N_CORES = 8
B, D, H, W, K = 64, 256, 32, 32, 1024
HW = H * W                      # 1024
BPC = B // N_CORES              # 8 batch items per core
NT = BPC * HW // 128            # 64 tiles of 128 rows per core

_cache = {}


def build_kernel():
    nc = bacc.Bacc()
    xg = nc.declare_dram_parameter("xg", [BPC, D, HW], F32R, isOutput=False)
    emb = nc.declare_dram_parameter("emb", [D, K], F32R, isOutput=False)
    embt = nc.declare_dram_parameter("embt", [K, D], F32, isOutput=False)
    nb2bc = nc.declare_dram_parameter("nb2bc", [128, K], F32, isOutput=False)
    iotabc = nc.declare_dram_parameter("iotabc", [128, K], F32, isOutput=False)

    outq = nc.declare_dram_parameter("outq", [BPC, D, HW], F32, isOutput=True)
    argm = nc.declare_dram_parameter("argm", [NT, 128], I32, isOutput=True)

    with tile.TileContext(nc) as tc, ExitStack() as ctx:
        const = ctx.enter_context(tc.tile_pool(name="const", bufs=1))
        xin = ctx.enter_context(tc.tile_pool(name="xin", bufs=4))
        work = ctx.enter_context(tc.tile_pool(name="work", bufs=4))
        gp = ctx.enter_context(tc.tile_pool(name="gp", bufs=4))
        ps = ctx.enter_context(tc.tile_pool(name="ps", bufs=3, space="PSUM"))
        pst = ctx.enter_context(tc.tile_pool(name="pst", bufs=2, space="PSUM"))

        e0 = const.tile([128, K], F32R, tag="e0")
        nc.sync.dma_start(e0[:], emb[0:128, :])
        e1 = const.tile([128, K], F32R, tag="e1")
        nc.sync.dma_start(e1[:], emb[128:256, :])
        nb2bc_t = const.tile([128, K], F32, tag="nb2bc")
        nc.sync.dma_start(nb2bc_t[:], nb2bc[:])
        iota_t = const.tile([128, K], F32, tag="iota")
        nc.sync.dma_start(iota_t[:], iotabc[:])
        ident = const.tile([128, 128], F32, tag="ident")
        make_identity(nc, ident[:])
        acc_all = const.tile([128, NT], F32, tag="acc_all")

        # One-time PSUM warmup: set has_written on both "p" slots so the
        # ACT bias-prefill + matmul(start=False) accumulation pattern works
        # from the first tile (has_written is only touched by TensorE).
        for _ in range(3):
            wp = ps.tile([128, K], F32, tag="p")
            for h in range(2):
                ks = slice(512 * h, 512 * h + 512)
                nc.tensor.matmul(wp[:, ks], nb2bc_t[0:1, 0:128], nb2bc_t[0:1, ks],
                                 start=True, stop=True, skip_group_check=True)

        for i in range(NT):
            b, t = divmod(i, HW // 128)
            hws = slice(128 * t, 128 * t + 128)

            xt = xin.tile([128, 2, 128], F32R, tag="xt")
            nc.sync.dma_start(xt[:], xg[b, :, hws].rearrange("(c p) w -> p c w", p=128))

            p = ps.tile([128, K], F32, tag="p")
            nc.scalar.copy(p[:], nb2bc_t[:])
            for h in range(2):
                ks = slice(512 * h, 512 * h + 512)
                nc.tensor.matmul(p[:, ks], xt[:, 0, :], e0[:, ks], start=False,
                                 stop=False, skip_group_check=True)
                nc.tensor.matmul(p[:, ks], xt[:, 1, :], e1[:, ks], start=False,
                                 stop=True, skip_group_check=True)

            m_t = work.tile([128, 1], F32, tag="m")
            nc.vector.reduce_max(m_t[:], p[:], axis=mybir.AxisListType.X)
            sttrash = work.tile([128, K], F16, tag="sttrash")
            nc.vector.scalar_tensor_tensor(
                out=sttrash[:], in0=p[:], scalar=m_t[:], in1=iota_t[:],
                op0=mybir.AluOpType.is_ge, op1=mybir.AluOpType.mult,
                accum_out=acc_all[:, i:i + 1])
            idx_t = work.tile([128, 1], I32, tag="idx")
            nc.vector.tensor_copy(idx_t[:], acc_all[:, i:i + 1])

            g = gp.tile([128, D], F32, tag="g")
            nc.gpsimd.indirect_dma_start(
                out=g[:], out_offset=None, in_=embt[:],
                in_offset=bass.IndirectOffsetOnAxis(ap=idx_t[:, 0:1], axis=0),
                bounds_check=K - 1, oob_is_err=False)

            for c in range(2):
                pt = pst.tile([128, 128], F32, tag="pt")
                nc.tensor.transpose(pt[:], g[:, 128 * c:128 * c + 128], ident[:])
                q_sb = work.tile([128, 128], F32, tag="q_sb")
                nc.scalar.copy(q_sb[:], pt[:])
                nc.sync.dma_start(outq[b, 128 * c:128 * c + 128, hws], q_sb[:])

        # argmin output: transpose ACC (128 x 64) -> (64 x 128), cast i32, DMA out
        pacc = pst.tile([64, 128], F32, tag="pt")
        nc.tensor.transpose(pacc[:], acc_all[:], ident[:])
        acct_i = work.tile([64, 128], I32, tag="acct")
        nc.vector.tensor_copy(acct_i[:], pacc[:])
        nc.gpsimd.dma_start(argm[:], acct_i[:])

    nc.compile()
    return nc


def _get_kernel():
    if "nc" not in _cache:
        _cache["nc"] = build_kernel()
    return _cache["nc"]


def kernel(x: np.ndarray, emb: np.ndarray):
    x = np.ascontiguousarray(x, dtype=np.float32)
    emb = np.ascontiguousarray(emb, dtype=np.float32)
    assert x.shape == (B, D, H, W) and emb.shape == (D, K)

    nc = _get_kernel()

    embt = np.ascontiguousarray(emb.T)
    nb2 = (-0.5 * (emb.astype(np.float64) ** 2).sum(0)).astype(np.float32)
    nb2bc = np.broadcast_to(nb2[None, :], (128, K)).copy()
    iotabc = np.broadcast_to(np.arange(K, dtype=np.float32)[None, :], (128, K)).copy()

    xr = x.reshape(B, D, HW)
    in_maps = []
    for c in range(N_CORES):
        in_maps.append(dict(
            xg=np.ascontiguousarray(xr[c * BPC:(c + 1) * BPC]),
            emb=emb, embt=embt, nb2bc=nb2bc, iotabc=iotabc,
        ))

    results = run_bass_kernel_spmd(nc, in_maps, list(range(N_CORES))).results

    out = np.concatenate([r["outq"] for r in results], axis=0)   # (B, D, HW)
    out = out.reshape(B, D, H, W)
    am = np.concatenate([r["argm"].reshape(BPC, HW) for r in results], axis=0)
    am = am.reshape(B, H, W).astype(np.int32)
    return out, am
